# revision 38
# baseline (speedup 1.0000x reference)
"""BinActive(BN(x)) -> 3x3 Conv -> ReLU, data-parallel across 8 NeuronCores.

Strategy:
  - Shard batch (64) across 8 cores (8 samples each); replicate gamma/beta/W.
  - BN+sign collapses to a per-channel threshold:
      xn = (x - mean) * rsqrt(var+eps) * gamma + beta
      sign(xn) = Sign(gamma * x + (beta*sqrt(var+eps) - gamma*mean))
    so the kernel never materializes the normalized tensor.
  - Pass 1: per-core per-channel bn_stats/bn_aggr over the batch shard, then a
    2KB AllReduce of (mean, E[x^2]) partials for exact batch statistics.
  - Pass 2: binarize into zero-padded flat bf16 tiles [128ch, 58*58] (ScalarE
    Sign with per-partition scale/bias), plus an fp8e4 shadow copy with the two
    ci-halves stacked ([128, 2, 3376], DVE convert; +-1 is exact in fp8).
    Implicit-GEMM conv per output chunk of 8 padded rows (464 px): FP8T of the
    9 taps run as fp8 DoubleRow matmuls (contraction 256 = both ci halves in
    one instruction, 2 elem/cycle), the rest as bf16 matmuls (one per ci half).
    All accumulate into one [128, 464] PSUM bank; pad columns carry junk that
    the evac discards. ReLU on the DVE evac; DMA to HBM.
  - Precision: fp8(e4m3) weight quantization on 4/9 taps gives rel err
    1.79e-2 measured end-to-end on HW vs the 2e-2 gate (deterministic seed,
    so this reproduces exactly); bf16 weights on the remaining taps add ~1e-3
    in quadrature. Stats/thresholds stay exact f32.
"""

import numpy as np

import concourse.bass as bass
import concourse.mybir as mybir
import concourse.tile as tile
from concourse import bacc
from concourse.bass_utils import run_bass_kernel_spmd

N_CORES = 8
N_PER = 8          # batch samples per core
C = 256            # input channels
CO = 256           # output channels
H = 56
Wsp = 56
KS = 3
EPS = 1e-5
HP = H + 2         # padded height
WP = Wsp + 2       # padded width
NCH = C // 128     # channel halves (2)
NCO = CO // 128    # out-channel halves (2)
ROWS = 8           # output rows per matmul chunk
CHUNKS = H // ROWS # 7
FLAT = HP * WP     # 3364 padded pixels
FREE = ROWS * WP   # 464 contiguous moving elements per matmul
X8P = 3376         # fp8 tile pitch per ci-half (FLAT padded to %16 for DoubleRow)

f32 = mybir.dt.float32
f32r = mybir.dt.float32r
bf16 = mybir.dt.bfloat16
fp8e4 = mybir.dt.float8e4

import os as _os

# moving/stationary dtype for the non-fp8 taps: bf16 measured ~38us/exec faster
# than f32r (FWL halves the inline weight-load; see microbench V0 vs V1)
USE_F32R = _os.environ.get("KERNEL_F32R", "0") == "1"
# number of conv taps (of 9) computed in fp8e4 DoubleRow; each covers both
# ci halves in one matmul at 2 elem/cycle. rel err ~= 2.64e-2 * sqrt(t/9).
FP8T = int(_os.environ.get("KERNEL_FP8T", "4"))
# bench decomposition: "all" | "p1" (stats only) | "p2" (conv only)
#   | "p2x" (conv, no y DMA) | "p2e" (conv, no evac/DMA: PE stream only)
BENCH_PART = _os.environ.get("KERNEL_PART", "all")

ALL_TAPS = [(kh, kw) for kh in range(KS) for kw in range(KS)]

_BUILT = None
LAST_RESULTS = None


def _build(bench_reps=0):
    # Bacc (not raw Bass): its compile() pass splits excess per-instruction
    # sync waits into EventSemaphore instructions (hardware allows at most
    # one wait on most instruction structs).
    #
    # bench_reps > 0 builds a timing variant: x/W/y live in internal DRAM
    # scratch (so the axon tunnel doesn't ship 400MB per call) and the whole
    # compute body runs bench_reps times inside a hardware For_i loop. The
    # AllReduce is hoisted out (collectives can't sit in control flow); an
    # artificial pass1->threshold dependency keeps the loop body's critical
    # path shaped like the real kernel.
    from contextlib import ExitStack, nullcontext

    nc = bacc.Bacc("TRN2", debug=False, num_devices=N_CORES)

    gamma = nc.dram_tensor("gamma", [C], f32, kind="ExternalInput").ap()
    beta = nc.dram_tensor("beta", [C], f32, kind="ExternalInput").ap()
    if bench_reps:
        x = nc.dram_tensor("x_int", [N_PER, C, H, Wsp], f32).ap()
        Wd = nc.dram_tensor("W_int", [CO, C, KS, KS], f32).ap()
        y = nc.dram_tensor("y_int", [N_PER, CO, H, Wsp], f32).ap()
        ysum = nc.dram_tensor("ysum", [128, 1], f32, kind="ExternalOutput").ap()
    else:
        x = nc.dram_tensor("x", [N_PER, C, H, Wsp], f32, kind="ExternalInput").ap()
        Wd = nc.dram_tensor("W", [CO, C, KS, KS], f32, kind="ExternalInput").ap()
        y = nc.dram_tensor("y", [N_PER, CO, H, Wsp], f32, kind="ExternalOutput").ap()
        ysum = None

    mm_dt = f32r if USE_F32R else bf16
    fp8_taps = ALL_TAPS[:FP8T]
    bf_taps = [
        (ch, kh, kw)
        for ch in range(NCH)
        for (kh, kw) in ALL_TAPS
        if (kh, kw) not in fp8_taps
    ]

    with tile.TileContext(nc) as tc, ExitStack() as stk:
        const = stk.enter_context(tc.tile_pool(name="const", bufs=1))
        # 6 slots: pass-1 of the next bench iteration slot-waits on pass-2
        # consumers of this iteration; one extra slot moves that boundary a
        # full sample earlier so the stats DMA starts sooner under the conv
        xp = stk.enter_context(tc.tile_pool(name="xp", bufs=6))
        xbp = stk.enter_context(tc.tile_pool(name="xbp", bufs=1))
        op = stk.enter_context(tc.tile_pool(name="op", bufs=6))
        dp = stk.enter_context(tc.tile_pool(name="dram", bufs=1, space="DRAM"))
        # ---------------- init: constants ----------------
        ident = const.tile([128, 128], f32, name="ident")
        nc.gpsimd.memset(ident, 0.0)
        nc.gpsimd.affine_select(
            out=ident,
            in_=ident,
            compare_op=mybir.AluOpType.not_equal,
            fill=1.0,
            base=0,
            pattern=[[-1, 128]],
            channel_multiplier=1,
        )

        g_sb = []
        b_sb = []
        for ch in range(NCH):
            g_t = const.tile([128, 1], f32, name=f"g_{ch}")
            nc.sync.dma_start(out=g_t, in_=gamma[ch * 128:(ch + 1) * 128])
            b_t = const.tile([128, 1], f32, name=f"b_{ch}")
            nc.sync.dma_start(out=b_t, in_=beta[ch * 128:(ch + 1) * 128])
            g_sb.append(g_t)
            b_sb.append(b_t)

        # Load W [co, ci, kh, kw] contiguously, then PE-transpose each
        # [co128, ci128] tap into stationary [ci, co] tiles (bf16 for the
        # per-half taps, fp8e4 [ci, 2, co] pairs for the DoubleRow taps).
        w_sb = []
        for co2 in range(NCO):
            w_t = const.tile([128, C, KS, KS], f32, name=f"w_{co2}")
            nc.sync.dma_start(out=w_t, in_=Wd[co2 * 128:(co2 + 1) * 128])
            w_sb.append(w_t)

        wT = {}
        wT8 = {}
        for co2 in range(NCO):
            for kh, kw in fp8_taps:
                wT8[(co2, kh, kw)] = const.tile(
                    [128, 2, 128], fp8e4, name=f"wT8_{co2}_{kh}_{kw}"
                )
        # Transpose-mode matmuls lower to a single S3_LW instruction that
        # only carries ONE sync wait, so make sure no transpose ever needs
        # two: a dummy ident-transpose absorbs the GpSimd tick, and the
        # co2-interleaved order lets each W-load DMA tick be absorbed by a
        # transpose whose PSUM slot has no cross-engine WAR yet.
        # The init transposes get their own PSUM pool, closed before the
        # conv pool opens, so the conv can use all 8 banks.
        with tc.tile_pool(name="psi", bufs=1, space="PSUM") as psi:
            tp_d = psi.tile([128, 128], f32, name="tp_d", tag="tpd", bufs=1)
            nc.tensor.transpose(tp_d, ident, ident)

            for ch in range(NCH):
                for kh in range(KS):
                    for kw in range(KS):
                        for co2 in range(NCO):
                            tp = psi.tile(
                                [128, 128], f32, name="tp", tag="tp", bufs=2
                            )
                            nc.tensor.transpose(
                                tp,
                                w_sb[co2][:, ch * 128:(ch + 1) * 128, kh, kw],
                                ident,
                            )
                            if (kh, kw) in fp8_taps:
                                nc.vector.tensor_copy(
                                    out=wT8[(co2, kh, kw)][:, ch, :], in_=tp
                                )
                            else:
                                wt = const.tile(
                                    [128, 128], mm_dt,
                                    name=f"wT_{co2}_{ch}_{kh}_{kw}",
                                )
                                nc.vector.tensor_copy(out=wt, in_=tp)
                                wT[(co2, ch, kh, kw)] = wt

        ps = stk.enter_context(tc.tile_pool(name="ps", bufs=1, space="PSUM"))

        # Double-buffered padded binary tiles, flat [128, 3364+2] so each
        # matmul streams FREE=464 contiguous elements; borders memset to zero
        # once and never rewritten (binarize only touches the interior).
        xbf = [[None] * NCH for _ in range(2)]
        xb8 = [None, None]
        for i in range(2):
            for ch in range(NCH):
                xf = xbp.tile([128, FLAT + 2], mm_dt, name=f"xb_{i}_{ch}")
                # DVE (not GpSimd) so the first conv matmul's wait set
                # stays within the 2 sync-wait slots (ACT + DVE).
                # (memset doesn't speak f32r; bitcast keeps bits 0)
                nc.vector.memset(xf.bitcast(f32) if USE_F32R else xf, 0.0)
                xbf[i][ch] = xf
            if fp8_taps:
                x8 = xbp.tile([128, NCH, X8P], fp8e4, name=f"xb8_{i}")
                nc.vector.memset(x8.bitcast(f32), 0.0)
                xb8[i] = x8

        eps_t = const.tile([128, 1], f32, name="eps_t")
        nc.vector.memset(eps_t, EPS)
        zb_t = const.tile([128, 1], f32, name="zb_t")
        nc.vector.memset(zb_t, 0.0)

        st = []
        for ch in range(NCH):
            st_t = const.tile([128, N_PER * CHUNKS, 6], f32, name=f"st_{ch}")
            nc.vector.memset(st_t, 0.0)
            st.append(st_t)

        ccin = [dp.tile([128, 2], f32, name=f"ccin_{ch}") for ch in range(NCH)]
        ccout = [
            dp.tile([128, 2], f32, name=f"ccout_{ch}", addr_space="Shared")
            for ch in range(NCH)
        ]

        def all_reduce_stats(ch):
            # split per channel-half: AR(ch0) fires as soon as ch0's shard
            # stats are aggregated and hides under ch1's stats DMA
            nc.gpsimd.collective_compute(
                "AllReduce",
                mybir.AluOpType.add,
                replica_groups=[list(range(N_CORES))],
                ins=[ccin[ch][:]],
                outs=[ccout[ch][:]],
            )

        if bench_reps:
            # collectives can't live inside control flow; run the ARs on
            # (uninitialized) ccin before the timing loop
            for ch in range(NCH):
                all_reduce_stats(ch)

        # p2 x loads are row-split: two DMAs per tile so each binarize ACT
        # instruction waits on at most one DMA tick. p1 loads are issued
        # unsplit and alternate between the two HWDGE rings (SP + ACT
        # sequencers) — HWDGE DMAs drain FIFO per issuing engine, so one
        # ring serializes the ~1.4us per-DMA issue/completion overhead on
        # pass 1's critical path; ACT is otherwise idle during pass 1.
        RSPLIT = 28

        def load_x(n, ch, split=True, eng=None):
            x_t = xp.tile([128, H, Wsp], f32, name="x_t")
            splits = ((0, RSPLIT), (RSPLIT, H)) if split else ((0, H),)
            for r0, r1 in splits:
                (eng or nc.sync).dma_start(
                    out=x_t[:, r0:r1, :],
                    in_=x[n, ch * 128:(ch + 1) * 128, r0:r1, :],
                )
            return x_t

        loop_cm = tc.For_i(0, bench_reps, 1) if bench_reps else nullcontext()
        run_p1 = not (bench_reps and BENCH_PART in ("p2", "p2x", "p2e"))
        run_p2 = not (bench_reps and BENCH_PART == "p1")
        do_evac = not (bench_reps and BENCH_PART == "p2e")
        do_ydma = not (bench_reps and BENCH_PART in ("p2e", "p2x"))

        def emit_stats(ch):
            # one unsplit DMA per (n, ch) tile, alternating the two HWDGE
            # rings (gpsimd SWDGE measured ~35us slower end-to-end and the
            # DVE FIFO pins the stats tail after pass-2's evacs regardless)
            for n in range(N_PER):
                eng = nc.sync if n % 2 == 0 else nc.scalar
                x_t = load_x(n, ch, split=False, eng=eng)
                for g in range(CHUNKS):
                    nc.vector.bn_stats(
                        out=st[ch][:, n * CHUNKS + g, :],
                        in_=x_t[:, g * ROWS:(g + 1) * ROWS, :].rearrange(
                            "c a b -> c (a b)"
                        ),
                    )
            # pack (mean, E[x^2]) partials for the AllReduce
            mv = const.tile([128, 2], f32, name=f"mv_{ch}")
            nc.vector.bn_aggr(out=mv, in_=st[ch])
            me = const.tile([128, 2], f32, name=f"me_{ch}")
            nc.vector.tensor_copy(out=me[:, 0:1], in_=mv[:, 0:1])
            nc.vector.tensor_mul(me[:, 1:2], mv[:, 0:1], mv[:, 0:1])
            nc.vector.tensor_add(me[:, 1:2], me[:, 1:2], mv[:, 1:2])
            nc.sync.dma_start(out=ccin[ch], in_=me)
            return mv

        def emit_threshold(ch, mv):
            # thresholds: scale_c = gamma_c,
            #             bias_c = beta_c*s_c - gamma_c*mean_c
            gs = const.tile([128, 2], f32, name=f"gs_{ch}")
            nc.sync.dma_start(out=gs, in_=ccout[ch])
            mean_g = const.tile([128, 1], f32, name=f"mg_{ch}")
            nc.vector.tensor_scalar_mul(mean_g, gs[:, 0:1], 1.0 / N_CORES)
            var_g = const.tile([128, 1], f32, name=f"vg_{ch}")
            nc.vector.tensor_scalar_mul(var_g, gs[:, 1:2], 1.0 / N_CORES)
            msq = const.tile([128, 1], f32, name=f"msq_{ch}")
            nc.vector.tensor_mul(msq, mean_g, mean_g)
            nc.vector.tensor_sub(var_g, var_g, msq)
            nc.vector.tensor_add(var_g, var_g, eps_t)
            # sqrt(var+eps) on the DVE via magic-constant rsqrt + 3 Newton
            # steps (reaches f32 roundoff): ACT Sqrt lives in a different
            # activation table set than Sign/Relu, so using it would cost
            # two ~2.7us ACT_TABLE_LOADs per pass
            s_t = const.tile([128, 1], f32, name=f"s_{ch}")
            r_t = const.tile([128, 1], f32, name=f"r_{ch}")
            hv = const.tile([128, 1], f32, name=f"hv_{ch}")
            t2 = const.tile([128, 1], f32, name=f"t2_{ch}")
            mg = const.tile([128, 1], f32, name=f"mg2_{ch}")
            # f32 whose bits are 0x5F3759DF
            nc.vector.memset(mg, 1.3211836172961055e19)
            nc.vector.tensor_scalar_mul(hv, var_g, 0.5)
            i32 = mybir.dt.int32
            nc.vector.tensor_scalar(
                out=r_t.bitcast(i32),
                in0=var_g.bitcast(i32),
                scalar1=1,
                scalar2=None,
                op0=mybir.AluOpType.logical_shift_right,
            )
            nc.vector.tensor_sub(
                r_t.bitcast(i32), mg.bitcast(i32), r_t.bitcast(i32)
            )
            for _ in range(3):
                nc.vector.tensor_mul(t2, r_t, r_t)
                nc.vector.tensor_mul(t2, t2, hv)
                # t2 = 1.5 - t2 via (t2 * -1) + 1.5
                nc.vector.tensor_scalar(
                    out=t2,
                    in0=t2,
                    scalar1=-1.0,
                    scalar2=1.5,
                    op0=mybir.AluOpType.mult,
                    op1=mybir.AluOpType.add,
                )
                nc.vector.tensor_mul(r_t, r_t, t2)
            nc.vector.tensor_mul(s_t, var_g, r_t)
            nb = const.tile([128, 1], f32, name=f"nb_{ch}")
            nc.vector.tensor_mul(nb, b_sb[ch], s_t)
            gm = const.tile([128, 1], f32, name=f"gm_{ch}")
            nc.vector.tensor_mul(gm, g_sb[ch], mean_g)
            nc.vector.tensor_sub(nb, nb, gm)
            if bench_reps and mv is not None:
                # nbias += 0*mv: restores the pass1 -> binarize
                # critical-path edge the hoisted AR would provide
                z_t = const.tile([128, 1], f32, name=f"z_{ch}")
                nc.vector.tensor_scalar_mul(z_t, mv[:, 0:1], 0.0)
                nc.vector.tensor_add(nb, nb, z_t)
            return nb

        with loop_cm:
            # ---------------- pass 1: batch stats ----------------
            # channel-major with a split AllReduce: AR(ch0) + threshold(ch0)
            # hide under ch1's stats DMA. threshold(ch0) is emitted AFTER
            # ch1's loads so its AR0-gated Sqrt doesn't head-of-line-block
            # ch1's ACT-ring DMA issues.
            nbias = [None] * NCH
            if run_p1:
                mv0 = emit_stats(0)
                if not bench_reps:
                    all_reduce_stats(0)
                mv1 = emit_stats(1)
                nbias[0] = emit_threshold(0, mv0)
                if not bench_reps:
                    all_reduce_stats(1)
                nbias[1] = emit_threshold(1, mv1)
            else:
                for ch in range(NCH):
                    nbias[ch] = emit_threshold(ch, None)

            # ---------------- pass 2: binarize + conv ----------------
            # per-chunk matmul schedule: fp8 DoubleRow taps interleaved
            # between bf16 per-half taps (spreads the 256-column fp8 weight
            # loads between bf16 streams). A DoubleRow tap goes FIRST: its
            # start=True write covers the full 464-wide PSUM region, so the
            # pad columns the strided bf16 outputs skip are still cleared.
            sched = []
            bi = 0
            for i, t8 in enumerate(fp8_taps):
                sched.append(("f8", t8))
                sched += [("bf", t) for t in bf_taps[bi:bi + 2]]
                bi += 2
            sched += [("bf", t) for t in bf_taps[bi:]]
            if not fp8_taps:
                sched = [("bf", t) for t in bf_taps]

            def emit_binarize(n):
                buf = n % 2
                for ch in range(NCH):
                    # alternate the two HWDGE rings for p2 loads as well;
                    # the ~2us per-DMA issue+completion cost on one FIFO
                    # ring otherwise approaches the PE-stream time. Unsplit:
                    # both binarize ACT instructions wait on the same single
                    # DMA tick (one wait each), and 16 fewer ring issues per
                    # pass free HWDGE head-time for the next iteration's
                    # stats loads.
                    x_t = load_x(
                        n, ch, split=False,
                        eng=nc.sync if (n * NCH + ch) % 2 == 0 else nc.scalar,
                    )
                    xbv = xbf[buf][ch][:, 0:FLAT].rearrange(
                        "c (h w) -> c h w", h=HP
                    )
                    # one fused binarize per tile: the unsplit x-load is a
                    # single DMA tick, so splitting by rows no longer helps
                    # any wait-set, and every conv matmul then waits on at
                    # most ONE binarize tick
                    nc.scalar.activation(
                        out=xbv[:, 1:1 + H, 1:1 + Wsp],
                        in_=x_t,
                        func=mybir.ActivationFunctionType.Sign,
                        bias=nbias[ch],
                        scale=g_sb[ch],
                    )
                    if fp8_taps:
                        # fp8 shadow of the whole padded tile (+-1 and 0
                        # are exact), one fused copy per tile
                        src = xbf[buf][ch][:, 0:FLAT]
                        nc.vector.tensor_copy(
                            out=xb8[buf][:, ch, 0:FLAT],
                            in_=src.bitcast(f32) if USE_F32R else src,
                        )

            last_o = None
            if run_p2:
                emit_binarize(0)
            for n in range(N_PER if run_p2 else 0):
                # software-pipeline: emit the NEXT sample's binarize before
                # this sample's conv groups. With the ReLU evac on ACT, the
                # strict-FIFO ACT queue would otherwise hold binarize(n+1)
                # hostage behind evac(n, last), serializing the samples.
                if n + 1 < N_PER:
                    emit_binarize(n + 1)
                buf = n % 2

                xbv3 = [
                    xbf[buf][ch][:, 0:FLAT].rearrange("c (h w) -> c h w", h=HP)
                    for ch in range(NCH)
                ]
                for co2 in range(NCO):
                    for ck in range(CHUNKS):
                        pst = ps.tile(
                            [128, FREE], f32, name="mm", tag="mm", bufs=7
                        )
                        pstv = pst.rearrange("c (r w) -> c r w", r=ROWS)[
                            :, :, 0:Wsp
                        ]
                        n_mm = len(sched)
                        for w_idx, (kind, tap) in enumerate(sched):
                            if kind == "bf":
                                ch, kh, kw = tap
                                # strided moving operand (8x56 rows) with a
                                # strided PSUM target: streams 448 instead of
                                # 464, measured at full rate (microbench V5)
                                nc.tensor.matmul(
                                    pstv,
                                    wT[(co2, ch, kh, kw)],
                                    xbv3[ch][
                                        :,
                                        ck * ROWS + kh:ck * ROWS + kh + ROWS,
                                        kw:kw + Wsp,
                                    ],
                                    start=(w_idx == 0),
                                    stop=(w_idx == n_mm - 1),
                                )
                            else:
                                kh, kw = tap
                                off = (ck * ROWS + kh) * WP + kw
                                nc.tensor.matmul(
                                    pst,
                                    wT8[(co2, kh, kw)],
                                    xb8[buf][:, :, off:off + FREE],
                                    start=(w_idx == 0),
                                    stop=(w_idx == n_mm - 1),
                                    perf_mode=mybir.MatmulPerfMode.DoubleRow,
                                )
                        # evac the full contiguous 464 on the ACT engine
                        # (Relu shares the loaded table set with Sign, and
                        # ScalarE has the fast PSUM read port); keeping the
                        # DVE clear of evacs lets the next iteration's
                        # bn_stats run as soon as their data lands instead
                        # of queuing behind 112 evacs on the strict-FIFO
                        # DVE queue. The DMA strides past the 2 pad columns
                        # per row for free.
                        if do_evac:
                            o_t = op.tile([128, FREE], f32, name="o_t")
                            nc.scalar.activation(
                                out=o_t,
                                in_=pst,
                                func=mybir.ActivationFunctionType.Relu,
                                bias=zb_t,
                                scale=1.0,
                            )
                            if do_ydma:
                                # y-writes go on the gpsimd SWDGE ring: they
                                # are latency-tolerant (op pool gives ~17us
                                # slack) and keeping them off the SP/ACT
                                # HWDGE rings lets the next iteration's
                                # stats loads start as soon as the last
                                # x-load clears (~87% of the span) instead
                                # of after the last y-write (100%).
                                y_eng = nc.gpsimd
                                y_eng.dma_start(
                                    out=y[
                                        n,
                                        co2 * 128:(co2 + 1) * 128,
                                        ck * ROWS:(ck + 1) * ROWS,
                                        :,
                                    ],
                                    in_=o_t.rearrange("c (r w) -> c r w", r=ROWS)[
                                        :, :, 0:Wsp
                                    ],
                                )
                            last_o = o_t

            if bench_reps:
                # tiny real output so the graph has a live result
                src = last_o[:, 0:1] if last_o is not None else st[0][:, 0, 0:1]
                nc.sync.dma_start(out=ysum, in_=src)

    nc.compile()
    return nc


def kernel(x, gamma, beta, W):
    global _BUILT, LAST_RESULTS
    import os
    # This container has no NTFF hook (antenv.axon_hooks); make sure a stray
    # BASS_TRACE env can never route us onto that path.
    os.environ["BASS_NEVER_TRACE"] = "1"
    if _BUILT is None:
        _BUILT = _build()
    nc = _BUILT

    x = np.ascontiguousarray(x, dtype=np.float32)
    gamma = np.ascontiguousarray(gamma, dtype=np.float32)
    beta = np.ascontiguousarray(beta, dtype=np.float32)
    W = np.ascontiguousarray(W, dtype=np.float32)

    in_maps = [
        {
            "x": x[c * N_PER:(c + 1) * N_PER],
            "gamma": gamma,
            "beta": beta,
            "W": W,
        }
        for c in range(N_CORES)
    ]
    res = run_bass_kernel_spmd(nc, in_maps, list(range(N_CORES)))
    LAST_RESULTS = res
    return np.concatenate([res.results[c]["y"] for c in range(N_CORES)], axis=0)


# ---------------------------------------------------------------------------
# Benchmarking: chain `reps` NEFF executions inside one jit (y_i -> x_{i+1})
# and difference wall times, isolating on-device exec from axon dispatch.
# Mirrors bass2jax.run_bass_via_pjrt's multi-core path, without donation.
# ---------------------------------------------------------------------------
def _collect_io(nc):
    import concourse.mybir as _mybir

    partition_name = nc.partition_id_tensor.name if nc.partition_id_tensor else None
    in_names, out_names, out_avals = [], [], []
    import jax

    for alloc in nc.m.functions[0].allocations:
        if not isinstance(alloc, _mybir.MemoryLocationSet):
            continue
        name = alloc.memorylocations[0].name
        if alloc.kind == "ExternalInput":
            if name != partition_name:
                in_names.append(name)
        elif alloc.kind == "ExternalOutput":
            out_names.append(name)
            out_avals.append(
                jax.core.ShapedArray(
                    tuple(alloc.tensor_shape), _mybir.dt.np(alloc.dtype)
                )
            )
    return partition_name, in_names, out_names, out_avals


def _make_bench_fn(nc):
    import jax
    from jax.experimental.shard_map import shard_map
    from jax.sharding import Mesh, PartitionSpec

    from concourse import bass2jax as b2j

    b2j.install_neuronx_cc_hook()
    partition_name, in_names, out_names, out_avals = _collect_io(nc)
    n_params = len(in_names)
    n_outs = len(out_names)
    all_in_names = tuple(
        in_names + out_names + ([partition_name] if partition_name else [])
    )

    def _body(*args):
        operands = list(args[:n_params + n_outs])
        if partition_name is not None:
            operands.append(b2j.partition_id_tensor())
        outs = b2j._bass_exec_p.bind(
            *operands,
            out_avals=tuple(out_avals),
            in_names=all_in_names,
            out_names=tuple(out_names),
            lowering_input_output_aliases=(),
            sim_require_finite=True,
            sim_require_nnan=True,
            nc=nc,
        )
        return tuple(outs)

    devices = jax.devices()[:N_CORES]
    mesh = Mesh(np.asarray(devices), ("core",))
    in_specs = (PartitionSpec("core"),) * (n_params + n_outs)
    out_specs = (PartitionSpec("core"),) * n_outs
    fn = jax.jit(
        shard_map(_body, mesh=mesh, in_specs=in_specs, out_specs=out_specs,
                  check_rep=False),
        keep_unused=True,
    )
    return fn, in_names, out_names, out_avals


AR_FLOOR_NS = 12_000  # 8-core 2KB AllReduce floor, excluded from the bench loop


def _time_bench_variant(reps, gamma, beta, iters):
    import time

    import jax

    nc = _build(bench_reps=reps)
    fn, in_names, out_names, out_avals = _make_bench_fn(nc)
    per_core = {
        "gamma": np.ascontiguousarray(gamma, dtype=np.float32),
        "beta": np.ascontiguousarray(beta, dtype=np.float32),
    }
    args = [
        np.concatenate([per_core[name]] * N_CORES, axis=0) for name in in_names
    ]
    for av in out_avals:
        args.append(np.zeros((N_CORES * av.shape[0], *av.shape[1:]), av.dtype))
    args = [jax.device_put(a) for a in args]

    t0 = time.perf_counter()
    jax.block_until_ready(fn(*args))
    print(f"  reps={reps}: first call (compile+run) {time.perf_counter()-t0:.1f}s",
          flush=True)
    walls = []
    for _ in range(iters):
        t0 = time.perf_counter()
        jax.block_until_ready(fn(*args))
        walls.append(time.perf_counter() - t0)
    walls.sort()
    med = walls[len(walls) // 2]
    print(f"  reps={reps}: wall min {walls[0]*1e3:.1f} med {med*1e3:.1f} ms "
          f"(n={iters})", flush=True)
    return walls


def bench_exec_ns(x, gamma, beta, W, reps_lo=16, reps_hi=528, iters=12):
    """Estimate per-execution on-device time by looping the whole compute
    body (minus the tiny AllReduce) inside a hardware For_i loop and
    differencing two rep counts; axon RTT and the 16MB-ish constant I/O
    cancel in the difference. Returns ns (AR floor added back)."""
    import os

    os.environ["BASS_NEVER_TRACE"] = "1"

    w_lo = _time_bench_variant(reps_lo, gamma, beta, iters)
    w_hi = _time_bench_variant(reps_hi, gamma, beta, iters)
    med_lo = w_lo[len(w_lo) // 2]
    med_hi = w_hi[len(w_hi) // 2]
    t_iter = (med_hi - med_lo) / (reps_hi - reps_lo)
    t_iter_min = (w_hi[0] - w_lo[0]) / (reps_hi - reps_lo)
    print(f"  per-iteration: med-diff {t_iter*1e6:.1f} us, "
          f"min-diff {t_iter_min*1e6:.1f} us (+{AR_FLOOR_NS/1000:.0f} us AR)",
          flush=True)
    return int(t_iter * 1e9) + AR_FLOOR_NS
